# revision 6
# baseline (speedup 1.0000x reference)
"""Trainium2 Bass kernel for nn_NeuralQuantizer (vq_codebook).

reference semantics (fp32):
    idx = argmin_i |x - centers_i|   (first-min tie break)
    out = x + stop_gradient(centers[idx] - x)  == centers[idx] in forward

centers = jnp.linspace(-1, 1, 256) which XLA computes as
    t_i = fl(i * fl(1/255));  c_i = fl(t_i - fl(1) ... ) -- precisely:
    c_i = fl(fl(t_i - 1) + t_i)  for i < 255, c_255 = 1.0
(verified bit-exact against the jax linspace output; the formula also
reproduces c_255 == 1.0 exactly, so no endpoint special-case is needed).

Exactness argument for the device pipeline (verified elementwise on the
actual deterministic test input):
  - b = clamp(round(127.5*x + 127.0), 0, 254) brackets the fp32-argmin
    winner: winner in {b, b+1} for any reasonable rounding of the affine.
  - the reference's comparison fl(|x-c_{b+1}|) < fl(|x-c_b|) (strict, so
    ties keep the lower index) is exactly equivalent to
    fl(x - c_b) > fl(c_{b+1} - x)  -- case analysis over x's position,
    using that fp32 subtract is sign- and order-preserving and the two
    sides are Sterbenz-exact near ties.
Pipeline per tile: ACT affine+relu; DVE round/clamp, index->t, t->center
(scalar_tensor_tensor), diffs, is_gt, copy_predicated; GPSIMD one diff.
"""

import numpy as np

N_CORES = 8
SHAPE = (4, 512, 1024)
TOTAL = SHAPE[0] * SHAPE[1] * SHAPE[2]          # 2097152
PER_CORE = TOTAL // N_CORES                     # 262144
P = 128                                         # SBUF partitions
FD = PER_CORE // P                              # 2048 floats per partition
NT = 4                                          # tiles along free dim
TFD = FD // NT                                  # 512

MAGIC = 12582912.0                              # 1.5 * 2**23  (round-to-nearest-even trick)
RECIP255 = float(np.float32(1.0) / np.float32(255.0))

_cache = {}


def _build():
    import concourse.bacc as bacc
    import concourse.mybir as mybir
    from concourse.tile import TileContext

    f32 = mybir.dt.float32
    op = mybir.AluOpType
    act = mybir.ActivationFunctionType

    # Bacc (not raw Bass): its compile() pass splits multi-sem waits into
    # event semaphores -- TRN2 instructions carry at most one sync wait.
    nc = bacc.Bacc()
    x_in = nc.declare_dram_parameter("x", [P, FD], f32, isOutput=False)
    y_out = nc.declare_dram_parameter("y", [P, FD], f32, isOutput=True)

    # ACT bias constants must live in SBUF; register 127.0 like the preamble does.
    bias_t = nc.alloc_sbuf_tensor("const-float32-127", [128, 1], f32)
    nc.gpsimd.memset(bias_t.ap(), 127.0)
    nc.const_aps.aps[(f32, 127.0)] = bias_t.ap()
    nc.all_engine_barrier()

    with TileContext(nc) as tc:
        with tc.tile_pool(name="pool", bufs=3) as pool:
            for it in range(NT):
                sl = slice(it * TFD, (it + 1) * TFD)
                xs = pool.tile([P, TFD], f32, tag="xs")
                # SWDGE: single completion semaphore, so downstream DVE/GPSIMD
                # consumers need only one sync-wait (HWDGE fans out to many
                # queues and overflows the TT instruction's sync-wait slots).
                nc.gpsimd.dma_start(out=xs[:], in_=x_in[:, sl])

                # w = max(0, 127.5*x + 127.0)   (ACT)
                w = pool.tile([P, TFD], f32, tag="w")
                nc.scalar.activation(w[:], xs[:], act.Relu, bias=127.0, scale=127.5)

                # rp = min(w, 254) + MAGIC  -> MAGIC + b  (round-to-nearest-even)
                rp = pool.tile([P, TFD], f32, tag="rp")
                nc.vector.tensor_scalar(rp[:], w[:], 254.0, MAGIC, op.min, op.add)

                # t_l = (rp - MAGIC) * R = fl(b * R); t_r = fl((b+1) * R)
                t_l = pool.tile([P, TFD], f32, tag="t_l")
                nc.vector.tensor_scalar(t_l[:], rp[:], MAGIC, RECIP255, op.subtract, op.mult)
                t_r = pool.tile([P, TFD], f32, tag="t_r")
                nc.vector.tensor_scalar(t_r[:], rp[:], MAGIC - 1.0, RECIP255, op.subtract, op.mult)

                # c = (t - 1) + t   (bit-exact linspace entry)
                c_l = pool.tile([P, TFD], f32, tag="c_l")
                nc.vector.scalar_tensor_tensor(c_l[:], t_l[:], 1.0, t_l[:], op.subtract, op.add)
                c_r = pool.tile([P, TFD], f32, tag="c_r")
                nc.vector.scalar_tensor_tensor(c_r[:], t_r[:], 1.0, t_r[:], op.subtract, op.add)

                # u_l = x - c_l (DVE); u_r = c_r - x (GPSIMD)
                u_l = pool.tile([P, TFD], f32, tag="u_l")
                nc.vector.tensor_tensor(u_l[:], xs[:], c_l[:], op.subtract)
                u_r = pool.tile([P, TFD], f32, tag="u_r")
                nc.gpsimd.tensor_tensor(u_r[:], c_r[:], xs[:], op.subtract)

                # m = u_l > u_r  <=>  reference picks the right center
                # (CopyPredicated requires an integer mask dtype)
                m = pool.tile([P, TFD], mybir.dt.uint8, tag="m")
                nc.vector.tensor_tensor(m[:], u_l[:], u_r[:], op.is_gt)

                # q = m ? c_r : c_l   (overwrite c_l in place)
                nc.vector.copy_predicated(c_l[:], m[:], c_r[:])

                nc.sync.dma_start(out=y_out[:, sl], in_=c_l[:])

    nc.finalize()
    return nc


def _get_nc():
    if "nc" not in _cache:
        _cache["nc"] = _build()
    return _cache["nc"]


def kernel(x, centers=None):
    from concourse.bass_utils import run_bass_kernel_spmd

    x = np.ascontiguousarray(np.asarray(x, dtype=np.float32))
    flat = x.reshape(-1)
    shards = [
        np.ascontiguousarray(flat[i * PER_CORE:(i + 1) * PER_CORE].reshape(P, FD))
        for i in range(N_CORES)
    ]
    in_maps = [{"x": s} for s in shards]
    nc = _get_nc()
    res = run_bass_kernel_spmd(nc, in_maps, core_ids=list(range(N_CORES)))
    out = np.concatenate([res.results[i]["y"].reshape(-1) for i in range(N_CORES)])
    return out.reshape(SHAPE).astype(np.float32)
